# revision 2
# baseline (speedup 1.0000x reference)
"""Trainium2 Bass kernel for nn_NewtonLoss (segment_reduce).

Computes, for K refinement states over N atoms grouped into M molecules:
    sq[k,i]   = ||states_x[k,i,:] - x_target[i,:]||^2
    S[m,k]    = segment_sum(sq[k], molecule_id)
    per_state = sum_m valid_m * S[m,k]/c_m / V
    loss      = sum_k w_k * per_state_k        (w = normalized gamma powers)

Strategy (8-core SPMD, memory-bound, ~roofline):
  - Atoms sharded across 8 NeuronCores as overlapping fixed-size windows
    (SHARD = 250880 >= N/8 = 250000); each atom OWNED by exactly one core.
  - Host builds a single per-atom weight w_i = 1/count(molecule(i)) for
    owned atoms (0 otherwise) from molecule_id -- pure index metadata per
    the molecule-contiguous sharding hint -- and ships sqrt(w) replicated
    over xyz as f16.  All FP work on states/target runs on-device.
  - On device per core, per (tile, k-pair) unit:
      DMA  states f32->f16 cast (SWDGE, 5.9KB descriptors, ~358 GB/s)
      DVE  diff = st - tg          (f16 TT, 2x mode, K-broadcast target)
      DVE  wd   = diff * sqrt(w)   (f16 TT, 2x mode, K-broadcast weights)
      ACT  Square(wd) with accum_out -> acc[P, tile*K+k]
    The single ACT op fuses square + xyz-sum + atom-sum: no scan, no
    strided adds, no separate multiply-reduce.
  - Host sums the tiny per-core accumulators into the final scalar.
"""

import os
import sys

import numpy as np

for _p in ("/opt/trn_rl_repo",):
    if os.path.isdir(_p) and _p not in sys.path:
        sys.path.insert(0, _p)

import concourse.bacc as bacc  # noqa: E402
import concourse.bass as bass  # noqa: E402
import concourse.tile as tile  # noqa: E402
from concourse import mybir  # noqa: E402

GAMMA = 0.7
NCORES = 8
P = 128  # partitions

# Full-problem geometry (N = 2_000_000 atoms):
K_FULL = 8
R_FULL = 490           # atoms per partition-row per tile
NTILES_FULL = 4        # SHARD = 4 * 128 * 490 = 250880 >= 250000
KP = 2                 # states per pipeline unit (k-pair)


def build_program(K=K_FULL, ntiles=NTILES_FULL, R=R_FULL, reps=1,
                  stbufs=6, dfbufs=2, sqbufs=2):
    """Build the single-core Bass program (run SPMD on all cores)."""
    RD = R * 3
    TILE = P * R
    SHARD = ntiles * TILE
    NKP = K // KP
    f32 = mybir.dt.float32
    f16 = mybir.dt.float16

    nc = bacc.Bacc("TRN2", target_bir_lowering=False, debug=False,
                   num_devices=1)
    states = nc.dram_tensor("states", [K, SHARD, 3], f32,
                            kind="ExternalInput").ap()
    target = nc.dram_tensor("target", [SHARD, 3], f32,
                            kind="ExternalInput").ap()
    w3d = nc.dram_tensor("w3", [SHARD, 3], f16, kind="ExternalInput").ap()
    accd = nc.dram_tensor("acc", [P, ntiles * K], f32,
                          kind="ExternalOutput").ap()

    # atom i lives at (tile t, partition p, row-pos r): i = t*TILE + p*R + r
    st_v = states.rearrange("k (t p r) d -> t p k (r d)", t=ntiles, p=P)
    tg_v = target.rearrange("(t p r) d -> p t (r d)", t=ntiles, p=P)
    w3_v = w3d.rearrange("(t p r) d -> p t (r d)", t=ntiles, p=P)

    def bcast(apv, n):
        """View [P, RD] slice as [P, n, RD] broadcast over the middle dim."""
        return bass.AP(tensor=apv.tensor, offset=apv.offset,
                       ap=[list(apv.ap[0]), [0, n], list(apv.ap[-1])])

    with tile.TileContext(nc) as tc:
        with (
            tc.tile_pool(name="singles", bufs=1) as singles,
            tc.tile_pool(name="tgp", bufs=2) as tgp,
            tc.tile_pool(name="stp", bufs=stbufs) as stp,
            tc.tile_pool(name="dfp", bufs=dfbufs) as dfp,
            tc.tile_pool(name="sqp", bufs=sqbufs) as sqp,
        ):
            w3_all = singles.tile([P, ntiles, RD], f16)
            nc.sync.dma_start(out=w3_all, in_=w3_v)
            acc = singles.tile([P, ntiles * K], f32)

            for _rep in range(reps):
                for t in range(ntiles):
                    tg_t = tgp.tile([P, RD], f16)
                    nc.gpsimd.dma_start(out=tg_t, in_=tg_v[:, t, :])
                    tgb = bcast(tg_t, KP)
                    w3b = bcast(w3_all[:, t, :], KP)
                    for kp in range(NKP):
                        k0 = kp * KP
                        st = stp.tile([P, KP, RD], f16)
                        nc.gpsimd.dma_start(
                            out=st, in_=st_v[t][:, k0:k0 + KP, :])
                        diff = dfp.tile([P, KP, RD], f16)
                        nc.vector.tensor_sub(diff, st, tgb)
                        nc.vector.tensor_mul(st, diff, w3b)
                        for kk in range(KP):
                            sq = sqp.tile([P, RD], f16)
                            slot = t * K + k0 + kk
                            nc.scalar.activation(
                                sq, st[:, kk, :],
                                mybir.ActivationFunctionType.Square,
                                accum_out=acc[:, slot:slot + 1])
            nc.sync.dma_start(out=accd, in_=acc)
    nc.compile()
    return nc


def host_prep(states_x, x_target, molecule_id, num_molecules,
              ncores=NCORES, K=K_FULL, ntiles=NTILES_FULL, R=R_FULL):
    """Shard inputs into per-core windows; build sqrt-weight vectors.

    Returns (in_maps, V) where in_maps[c] are the named inputs for core c.
    """
    TILE = P * R
    SHARD = ntiles * TILE
    N = molecule_id.shape[0]
    M = int(num_molecules)
    assert N % ncores == 0
    OWN = N // ncores
    assert SHARD >= OWN, (SHARD, OWN)

    ids = np.asarray(molecule_id).astype(np.int64)
    counts = np.bincount(ids, minlength=M)
    V = int((counts > 0).sum())
    inv_c = np.zeros(M, np.float64)
    nz = counts > 0
    inv_c[nz] = 1.0 / counts[nz]

    states_x = np.asarray(states_x)
    x_target = np.asarray(x_target)

    in_maps = []
    for c in range(ncores):
        S_c = 0 if ncores == 1 else (c * (N - SHARD)) // (ncores - 1)
        own_lo, own_hi = c * OWN - S_c, (c + 1) * OWN - S_c
        assert own_lo >= 0 and own_hi <= SHARD

        idw = ids[S_c:S_c + SHARD]
        pos = np.arange(SHARD, dtype=np.int64)
        owned = (pos >= own_lo) & (pos < own_hi)
        w = np.where(owned, inv_c[idw], 0.0)
        w3 = np.broadcast_to(np.sqrt(w).astype(np.float16)[:, None],
                             (SHARD, 3))

        in_maps.append({
            "states": np.ascontiguousarray(states_x[:, S_c:S_c + SHARD, :],
                                           dtype=np.float32),
            "target": np.ascontiguousarray(x_target[S_c:S_c + SHARD, :],
                                           dtype=np.float32),
            "w3": np.ascontiguousarray(w3),
        })
    return in_maps, V


def combine(results, V, K=K_FULL):
    """Sum per-core accumulators into the final scalar loss."""
    total = np.zeros(K, np.float64)
    for r in results:
        acc = np.asarray(r["acc"]).astype(np.float64)  # [P, ntiles*K]
        total += acc.reshape(P, -1, K).sum(axis=(0, 1))
    per_state = total / V
    w = GAMMA ** ((K - 1) - np.arange(K, dtype=np.float64))
    w = w / w.sum()
    return np.float32((w * per_state).sum())


class Runner:
    """Caches the compiled PJRT executable for repeated SPMD runs."""

    def __init__(self, nc, n_cores=NCORES, n_inner=1):
        import jax
        from jax.experimental.shard_map import shard_map
        from jax.sharding import Mesh, PartitionSpec
        from concourse import bass2jax, mybir as _mybir

        bass2jax.install_neuronx_cc_hook()
        self.jax = jax
        self.nc = nc
        self.n_cores = n_cores

        partition_name = (nc.partition_id_tensor.name
                          if nc.partition_id_tensor else None)
        in_names, out_names, out_avals, zero_outs = [], [], [], []
        for alloc in nc.m.functions[0].allocations:
            if not isinstance(alloc, _mybir.MemoryLocationSet):
                continue
            name = alloc.memorylocations[0].name
            if alloc.kind == "ExternalInput":
                if name != partition_name:
                    in_names.append(name)
            elif alloc.kind == "ExternalOutput":
                shape = tuple(alloc.tensor_shape)
                dtype = _mybir.dt.np(alloc.dtype)
                out_names.append(name)
                out_avals.append(jax.core.ShapedArray(shape, dtype))
                zero_outs.append(np.zeros(shape, dtype))
        self.in_names, self.out_names = in_names, out_names
        self.out_avals, self.zero_outs = out_avals, zero_outs
        n_params = len(in_names)
        all_in_names = list(in_names) + list(out_names)
        if partition_name is not None:
            all_in_names.append(partition_name)

        def _body(*args):
            ins = list(args[:n_params])
            cur_zeros = list(args[n_params:n_params + len(out_names)])
            extra = ([bass2jax.partition_id_tensor()]
                     if partition_name is not None else [])
            outs = tuple(cur_zeros)
            for _ in range(n_inner):
                outs = bass2jax._bass_exec_p.bind(
                    *ins, *outs, *extra,
                    out_avals=tuple(out_avals),
                    in_names=tuple(all_in_names),
                    out_names=tuple(out_names),
                    lowering_input_output_aliases=(),
                    sim_require_finite=True,
                    sim_require_nnan=True,
                    nc=nc,
                )
            return tuple(outs)

        devices = jax.devices()[:n_cores]
        assert len(devices) == n_cores
        self.mesh = Mesh(np.asarray(devices), ("core",))
        self.pspec = PartitionSpec("core")
        n_outs = len(out_names)
        in_specs = (self.pspec,) * (n_params + n_outs)
        out_specs = (self.pspec,) * n_outs
        donate = tuple(range(n_params, n_params + n_outs))
        self.fn = jax.jit(
            shard_map(_body, mesh=self.mesh, in_specs=in_specs,
                      out_specs=out_specs, check_rep=False),
            donate_argnums=donate, keep_unused=True)

    def concat_inputs(self, in_maps):
        return [np.concatenate([np.asarray(in_maps[c][n])
                                for c in range(self.n_cores)], axis=0)
                for n in self.in_names]

    def device_put(self, concat_in):
        from jax.sharding import NamedSharding
        sh = NamedSharding(self.mesh, self.pspec)
        return [self.jax.device_put(a, sh) for a in concat_in]

    def run_dev(self, dev_args):
        zeros = [np.zeros((self.n_cores * z.shape[0], *z.shape[1:]), z.dtype)
                 for z in self.zero_outs]
        out = self.fn(*dev_args, *zeros)
        return self.jax.block_until_ready(out)

    def run(self, in_maps):
        out_arrs = self.run_dev(self.device_put(self.concat_inputs(in_maps)))
        return [
            {n: np.asarray(out_arrs[i]).reshape(
                self.n_cores, *self.out_avals[i].shape)[c]
             for i, n in enumerate(self.out_names)}
            for c in range(self.n_cores)
        ]


_CACHE = {}


def get_runner(reps=1, n_inner=1, **kw):
    key = (reps, n_inner, tuple(sorted(kw.items())))
    if key not in _CACHE:
        nc = build_program(reps=reps, **kw)
        _CACHE[key] = Runner(nc, n_inner=n_inner)
    return _CACHE[key]


def kernel(states_x, x_target, molecule_id, num_molecules):
    runner = get_runner()
    in_maps, V = host_prep(states_x, x_target, molecule_id, num_molecules)
    results = runner.run(in_maps)
    return combine(results, V)


# revision 3
# speedup vs baseline: 1.0215x; 1.0215x over previous
"""Trainium2 Bass kernel for nn_NewtonLoss (segment_reduce).

Computes, for K refinement states over N atoms grouped into M molecules:
    sq[k,i]   = ||states_x[k,i,:] - x_target[i,:]||^2
    S[m,k]    = segment_sum(sq[k], molecule_id)
    per_state = sum_m valid_m * S[m,k]/c_m / V
    loss      = sum_k w_k * per_state_k        (w = normalized gamma powers)

Strategy (8-core SPMD, memory-bound, ~roofline):
  - Atoms sharded across 8 NeuronCores as overlapping fixed-size windows
    (SHARD = 250880 >= N/8 = 250000); each atom OWNED by exactly one core.
  - Host builds a single per-atom weight w_i = 1/count(molecule(i)) for
    owned atoms (0 otherwise) from molecule_id -- pure index metadata per
    the molecule-contiguous sharding hint -- and ships sqrt(w) replicated
    over xyz as f16.  All FP work on states/target runs on-device.
  - On device per core, per (tile, k-pair) unit:
      DMA  states f32->f16 cast (SWDGE, 5.9KB descriptors, ~358 GB/s)
      DVE  diff = st - tg          (f16 TT, 2x mode, K-broadcast target)
      DVE  wd   = diff * sqrt(w)   (f16 TT, 2x mode, K-broadcast weights)
      ACT  Square(wd) with accum_out -> acc[P, tile*K+k]
    The single ACT op fuses square + xyz-sum + atom-sum: no scan, no
    strided adds, no separate multiply-reduce.
  - Host sums the tiny per-core accumulators into the final scalar.
"""

import os
import sys

import numpy as np

for _p in ("/opt/trn_rl_repo",):
    if os.path.isdir(_p) and _p not in sys.path:
        sys.path.insert(0, _p)

import concourse.bacc as bacc  # noqa: E402
import concourse.bass as bass  # noqa: E402
import concourse.tile as tile  # noqa: E402
from concourse import mybir  # noqa: E402

GAMMA = 0.7
NCORES = 8
P = 128  # partitions

# Full-problem geometry (N = 2_000_000 atoms):
K_FULL = 8
R_FULL = 490           # atoms per partition-row per tile
NTILES_FULL = 4        # SHARD = 4 * 128 * 490 = 250880 >= 250000
KP = 2                 # states per pipeline unit (k-pair)


def build_program(K=K_FULL, ntiles=NTILES_FULL, R=R_FULL, reps=1,
                  stbufs=6, dfbufs=2, sqbufs=2):
    """Build the single-core Bass program (run SPMD on all cores)."""
    RD = R * 3
    TILE = P * R
    SHARD = ntiles * TILE
    NKP = K // KP
    f32 = mybir.dt.float32
    f16 = mybir.dt.float16

    nc = bacc.Bacc("TRN2", target_bir_lowering=False, debug=False,
                   num_devices=1)
    states = nc.dram_tensor("states", [K, SHARD, 3], f32,
                            kind="ExternalInput").ap()
    target = nc.dram_tensor("target", [SHARD, 3], f32,
                            kind="ExternalInput").ap()
    w3d = nc.dram_tensor("w3", [SHARD, 3], f16, kind="ExternalInput").ap()
    accd = nc.dram_tensor("acc", [P, ntiles * K], f32,
                          kind="ExternalOutput").ap()

    # atom i lives at (tile t, partition p, row-pos r): i = t*TILE + p*R + r
    st_v = states.rearrange("k (t p r) d -> t p k (r d)", t=ntiles, p=P)
    tg_v = target.rearrange("(t p r) d -> p t (r d)", t=ntiles, p=P)
    w3_v = w3d.rearrange("(t p r) d -> p t (r d)", t=ntiles, p=P)

    def bcast(apv, n):
        """View [P, RD] slice as [P, n, RD] broadcast over the middle dim."""
        return bass.AP(tensor=apv.tensor, offset=apv.offset,
                       ap=[list(apv.ap[0]), [0, n], list(apv.ap[-1])])

    with tile.TileContext(nc) as tc:
        with (
            tc.tile_pool(name="singles", bufs=1) as singles,
            tc.tile_pool(name="tgp", bufs=2) as tgp,
            tc.tile_pool(name="stp", bufs=stbufs) as stp,
            tc.tile_pool(name="dfp", bufs=dfbufs) as dfp,
            tc.tile_pool(name="sqp", bufs=sqbufs) as sqp,
        ):
            w3_all = singles.tile([P, ntiles, RD], f16)
            nc.sync.dma_start(out=w3_all, in_=w3_v)
            acc = singles.tile([P, ntiles * K], f32)

            for _rep in range(reps):
                for t in range(ntiles):
                    # finer units at the pipeline edges shrink head/tail
                    kp_sz = 1 if (t == 0 or t == ntiles - 1) else KP
                    tg_t = tgp.tile([P, RD], f16)
                    nc.gpsimd.dma_start(out=tg_t, in_=tg_v[:, t, :])
                    tgb = bcast(tg_t, kp_sz)
                    w3b = bcast(w3_all[:, t, :], kp_sz)
                    for kp in range(K // kp_sz):
                        k0 = kp * kp_sz
                        st = stp.tile([P, kp_sz, RD], f16)
                        nc.gpsimd.dma_start(
                            out=st, in_=st_v[t][:, k0:k0 + kp_sz, :])
                        diff = dfp.tile([P, kp_sz, RD], f16)
                        nc.vector.tensor_sub(diff, st, tgb)
                        nc.vector.tensor_mul(st, diff, w3b)
                        for kk in range(kp_sz):
                            sq = sqp.tile([P, RD], f16)
                            slot = t * K + k0 + kk
                            nc.scalar.activation(
                                sq, st[:, kk, :],
                                mybir.ActivationFunctionType.Square,
                                accum_out=acc[:, slot:slot + 1])
            nc.sync.dma_start(out=accd, in_=acc)
    nc.compile()
    return nc


def host_prep(states_x, x_target, molecule_id, num_molecules,
              ncores=NCORES, K=K_FULL, ntiles=NTILES_FULL, R=R_FULL):
    """Shard inputs into per-core windows; build sqrt-weight vectors.

    Returns (in_maps, V) where in_maps[c] are the named inputs for core c.
    """
    TILE = P * R
    SHARD = ntiles * TILE
    N = molecule_id.shape[0]
    M = int(num_molecules)
    assert N % ncores == 0
    OWN = N // ncores
    assert SHARD >= OWN, (SHARD, OWN)

    ids = np.asarray(molecule_id).astype(np.int64)
    counts = np.bincount(ids, minlength=M)
    V = int((counts > 0).sum())
    inv_c = np.zeros(M, np.float64)
    nz = counts > 0
    inv_c[nz] = 1.0 / counts[nz]

    states_x = np.asarray(states_x)
    x_target = np.asarray(x_target)

    in_maps = []
    for c in range(ncores):
        S_c = 0 if ncores == 1 else (c * (N - SHARD)) // (ncores - 1)
        own_lo, own_hi = c * OWN - S_c, (c + 1) * OWN - S_c
        assert own_lo >= 0 and own_hi <= SHARD

        idw = ids[S_c:S_c + SHARD]
        pos = np.arange(SHARD, dtype=np.int64)
        owned = (pos >= own_lo) & (pos < own_hi)
        w = np.where(owned, inv_c[idw], 0.0)
        w3 = np.broadcast_to(np.sqrt(w).astype(np.float16)[:, None],
                             (SHARD, 3))

        in_maps.append({
            "states": np.ascontiguousarray(states_x[:, S_c:S_c + SHARD, :],
                                           dtype=np.float32),
            "target": np.ascontiguousarray(x_target[S_c:S_c + SHARD, :],
                                           dtype=np.float32),
            "w3": np.ascontiguousarray(w3),
        })
    return in_maps, V


def combine(results, V, K=K_FULL):
    """Sum per-core accumulators into the final scalar loss."""
    total = np.zeros(K, np.float64)
    for r in results:
        acc = np.asarray(r["acc"]).astype(np.float64)  # [P, ntiles*K]
        total += acc.reshape(P, -1, K).sum(axis=(0, 1))
    per_state = total / V
    w = GAMMA ** ((K - 1) - np.arange(K, dtype=np.float64))
    w = w / w.sum()
    return np.float32((w * per_state).sum())


class Runner:
    """Caches the compiled PJRT executable for repeated SPMD runs."""

    def __init__(self, nc, n_cores=NCORES, n_inner=1):
        import jax
        from jax.experimental.shard_map import shard_map
        from jax.sharding import Mesh, PartitionSpec
        from concourse import bass2jax, mybir as _mybir

        bass2jax.install_neuronx_cc_hook()
        self.jax = jax
        self.nc = nc
        self.n_cores = n_cores

        partition_name = (nc.partition_id_tensor.name
                          if nc.partition_id_tensor else None)
        in_names, out_names, out_avals, zero_outs = [], [], [], []
        for alloc in nc.m.functions[0].allocations:
            if not isinstance(alloc, _mybir.MemoryLocationSet):
                continue
            name = alloc.memorylocations[0].name
            if alloc.kind == "ExternalInput":
                if name != partition_name:
                    in_names.append(name)
            elif alloc.kind == "ExternalOutput":
                shape = tuple(alloc.tensor_shape)
                dtype = _mybir.dt.np(alloc.dtype)
                out_names.append(name)
                out_avals.append(jax.core.ShapedArray(shape, dtype))
                zero_outs.append(np.zeros(shape, dtype))
        self.in_names, self.out_names = in_names, out_names
        self.out_avals, self.zero_outs = out_avals, zero_outs
        n_params = len(in_names)
        all_in_names = list(in_names) + list(out_names)
        if partition_name is not None:
            all_in_names.append(partition_name)

        def _body(*args):
            ins = list(args[:n_params])
            cur_zeros = list(args[n_params:n_params + len(out_names)])
            extra = ([bass2jax.partition_id_tensor()]
                     if partition_name is not None else [])
            outs = tuple(cur_zeros)
            for _ in range(n_inner):
                outs = bass2jax._bass_exec_p.bind(
                    *ins, *outs, *extra,
                    out_avals=tuple(out_avals),
                    in_names=tuple(all_in_names),
                    out_names=tuple(out_names),
                    lowering_input_output_aliases=(),
                    sim_require_finite=True,
                    sim_require_nnan=True,
                    nc=nc,
                )
            return tuple(outs)

        devices = jax.devices()[:n_cores]
        assert len(devices) == n_cores
        self.mesh = Mesh(np.asarray(devices), ("core",))
        self.pspec = PartitionSpec("core")
        n_outs = len(out_names)
        in_specs = (self.pspec,) * (n_params + n_outs)
        out_specs = (self.pspec,) * n_outs
        donate = tuple(range(n_params, n_params + n_outs))
        self.fn = jax.jit(
            shard_map(_body, mesh=self.mesh, in_specs=in_specs,
                      out_specs=out_specs, check_rep=False),
            donate_argnums=donate, keep_unused=True)

    def concat_inputs(self, in_maps):
        return [np.concatenate([np.asarray(in_maps[c][n])
                                for c in range(self.n_cores)], axis=0)
                for n in self.in_names]

    def device_put(self, concat_in):
        from jax.sharding import NamedSharding
        sh = NamedSharding(self.mesh, self.pspec)
        return [self.jax.device_put(a, sh) for a in concat_in]

    def run_dev(self, dev_args):
        zeros = [np.zeros((self.n_cores * z.shape[0], *z.shape[1:]), z.dtype)
                 for z in self.zero_outs]
        out = self.fn(*dev_args, *zeros)
        return self.jax.block_until_ready(out)

    def run(self, in_maps):
        out_arrs = self.run_dev(self.device_put(self.concat_inputs(in_maps)))
        return [
            {n: np.asarray(out_arrs[i]).reshape(
                self.n_cores, *self.out_avals[i].shape)[c]
             for i, n in enumerate(self.out_names)}
            for c in range(self.n_cores)
        ]


_CACHE = {}


def get_runner(reps=1, n_inner=1, **kw):
    key = (reps, n_inner, tuple(sorted(kw.items())))
    if key not in _CACHE:
        nc = build_program(reps=reps, **kw)
        _CACHE[key] = Runner(nc, n_inner=n_inner)
    return _CACHE[key]


def kernel(states_x, x_target, molecule_id, num_molecules):
    runner = get_runner()
    in_maps, V = host_prep(states_x, x_target, molecule_id, num_molecules)
    results = runner.run(in_maps)
    return combine(results, V)
